# revision 11
# baseline (speedup 1.0000x reference)
"""GQA attention (B=2, S=2048, D=2048, H=32, G=8, hd=64) on 8 TRN2 cores.

Sharding: core c owns (batch b=c//4, token block q0=512*(c%4)). The ACT
engine (256 exp instrs, ~280us) is the critical path, so the schedule is
built to start the exp stream at ~35us and keep it fed:

  - K tile 0 + V tiles 0,1 (vaug cols 0:260): replicated compute over
    all S (x^T streamed per 512-token chunk), interleaved per-chunk with
    Q chains and attention round 0 pair 0 so scores/exp start right
    after the first chunk lands.
  - K tile 1: replicated compute (pass B), interleaved into round 0.
  - V tiles 2,3 (vaug cols 260:520): replicated compute (pass C),
    interleaved into rounds 1-2 PE slack.
  - K tiles 2,3: TOKEN-SHARDED + one tiny AllGather (0.26 MB) per
    4-core batch group, kicked ~56us, needed ~180us (round 2).

Attention, softmax denominators (augmented-V ones column -> PSUM row 64
of the PV accumulator), and the output projection are fully local to the
core's 512 query tokens. Reciprocals 1/d run on DVE
(reciprocal_approx_fast) so ACT does pure exp with one table load.

PSUM: scores 2x[128,1024] (4 banks) + chain accumulators 2x[128,512]
(2) + PV accumulator [65,1024] (2, heads A|B side by side). The output
projection reuses all 8 banks as 8 held accumulators (2 tok-blocks per
psum scores tile) while wo streams through SBUF in 0.5MB chunks.
"""

import sys

sys.path.insert(0, "/opt/trn_rl_repo")

import numpy as np
import ml_dtypes

import concourse.bass as bass
import concourse.tile as tile
from concourse import bacc, mybir
from concourse.bass_utils import run_bass_kernel_spmd

BF16 = ml_dtypes.bfloat16
B, S, D = 2, 2048, 2048
H, G, HD = 32, 8, 64
DC = D // 128  # 16 dim chunks
N_CORES = 8
TOK = 512  # own tokens per core

_CACHE = {}


def _build():
    f32 = mybir.dt.float32
    bf16 = mybir.dt.bfloat16
    nc = bacc.Bacc("TRN2", target_bir_lowering=False, debug=False, num_devices=N_CORES)

    xt = nc.dram_tensor("xt", [128, DC, TOK], bf16, kind="ExternalInput").ap()
    xtf = nc.dram_tensor("xtf", [128, DC, S], bf16, kind="ExternalInput").ap()
    wq = nc.dram_tensor("wq", [128, DC, DC, 128], bf16, kind="ExternalInput").ap()
    wk = nc.dram_tensor("wk", [128, DC, 4, 128], bf16, kind="ExternalInput").ap()
    wv = nc.dram_tensor("wv", [128, DC, 512], bf16, kind="ExternalInput").ap()
    coso = nc.dram_tensor("coso", [128, TOK], bf16, kind="ExternalInput").ap()
    sino = nc.dram_tensor("sino", [128, TOK], bf16, kind="ExternalInput").ap()
    cosf = nc.dram_tensor("cosf", [128, S], bf16, kind="ExternalInput").ap()
    sinf = nc.dram_tensor("sinf", [128, S], bf16, kind="ExternalInput").ap()
    wo = nc.dram_tensor("wo", [128, DC, D], bf16, kind="ExternalInput").ap()
    out = nc.dram_tensor("out", [TOK, D], f32, kind="ExternalOutput").ap()

    Exp = mybir.ActivationFunctionType.Exp
    swap_mask = [i ^ 1 for i in range(32)]
    scale = float(1.0 / np.sqrt(HD))

    from contextlib import ExitStack
    with tile.TileContext(nc) as tc, ExitStack() as ctx:
        consts = ctx.enter_context(tc.tile_pool(name="consts", bufs=1))
        xtfp = ctx.enter_context(tc.tile_pool(name="xtfp", bufs=2))
        wqp = ctx.enter_context(tc.tile_pool(name="wqp", bufs=2))
        wop = ctx.enter_context(tc.tile_pool(name="wop", bufs=2))
        io = ctx.enter_context(tc.tile_pool(name="io", bufs=2))
        work = ctx.enter_context(tc.tile_pool(name="work", bufs=3))
        outw = ctx.enter_context(tc.tile_pool(name="outw", bufs=2))
        psum = ctx.enter_context(tc.tile_pool(name="psum", bufs=3, space="PSUM"))
        apsum = ctx.enter_context(tc.tile_pool(name="apsum", bufs=1, space="PSUM"))
        dram = ctx.enter_context(tc.tile_pool(name="dram", bufs=1, space="DRAM"))

        # ---- header loads, ordered to unlock: K-chunk0 -> Q0 -> V-chunk0
        wk_sb = consts.tile([128, DC, 4, 128], bf16, tag="wk")
        nc.sync.dma_start(out=wk_sb[:, :, 0:1, :], in_=wk[:, :, 0:1, :])
        cos_sb = consts.tile([128, TOK], bf16, tag="cos")
        nc.sync.dma_start(out=cos_sb[:], in_=coso[:])
        sin_sb = consts.tile([128, TOK], bf16, tag="sin")
        nc.sync.dma_start(out=sin_sb[:], in_=sino[:])
        xfa0 = xtfp.tile([128, DC, 512], bf16, tag="xf", name="xfa0")
        nc.sync.dma_start(out=xfa0[:], in_=xtf[:, :, 0:512])
        cosf_sb = consts.tile([128, S], bf16, tag="cosf")
        nc.sync.dma_start(out=cosf_sb[:], in_=cosf[:])
        sinf_sb = consts.tile([128, S], bf16, tag="sinf")
        nc.sync.dma_start(out=sinf_sb[:], in_=sinf[:])
        xt_sb = consts.tile([128, DC, TOK], bf16, tag="xt")
        nc.sync.dma_start(out=xt_sb[:], in_=xt[:])
        # wq0 is emitted by q_chain(0) below -> lands here in queue order
        wv_sb = consts.tile([128, DC, 512], bf16, tag="wv")
        wk23_loaded = []

        # preheat the exp table set (one ACT_TABLE_LOAD, early)
        preheat = io.tile([1, 64], bf16, tag="pre")
        nc.vector.memset(preheat[:], 0.0)
        nc.scalar.activation(preheat[:], preheat[:], Exp)

        kv_own = consts.tile([128, 1024], bf16, tag="kv")
        kt_sb = consts.tile([128, 4, S], bf16, tag="kt")
        vaug_sb = consts.tile([128, DC, 520], bf16, tag="vaug")
        dstage = consts.tile([97, 4096], f32, tag="dstage")
        nc.vector.memset(dstage[:], 1.0)
        ddram = dram.tile([4, 4096], f32, tag="dd", name="dd")
        # ones columns of augmented V (denominator accumulators)
        for col in (64, 129, 194, 259, 324, 389, 454, 519):
            nc.vector.memset(vaug_sb[:, :, col:col + 1], 1.0)

        def rope(ap, cs, sn):
            sw = io.tile([128, 512], bf16, tag="rsw")
            nc.vector.stream_shuffle(sw, ap, swap_mask)
            nc.vector.tensor_mul(sw, sw, sn)
            tmp = io.tile([128, 512], bf16, tag="rtmp")
            nc.vector.tensor_mul(tmp, ap, cs)
            nc.vector.tensor_add(ap, sw, tmp)

        def xf_dma(ssl, name):
            xf = xtfp.tile([128, DC, 512], bf16, tag="xf", name=name)
            nc.sync.dma_start(out=xf[:], in_=xtf[:, :, 512 * ssl:512 * (ssl + 1)])
            return xf

        def pass_K(t, ssl, xf):
            sl = slice(512 * ssl, 512 * (ssl + 1))
            pst = psum.tile([128, 1024], f32, tag="s", name=f"psk{t}_{ssl}")
            ps = pst[:, 0:512]
            for c in range(DC):
                nc.tensor.matmul(
                    ps, lhsT=wk_sb[:, c, t, :], rhs=xf[:, c, :],
                    start=(c == 0), stop=(c == DC - 1),
                )
            nc.vector.tensor_copy(kt_sb[:, t, sl], ps)
            rope(kt_sb[:, t, sl], cosf_sb[:, sl], sinf_sb[:, sl])

        def pass_V(ssl, xf):
            # full V projection for 512 tokens -> vaug cols 0:520
            for tbl in range(4):
                tb = 4 * ssl + tbl
                pst = psum.tile([128, 1024], f32, tag="s", name=f"psv{tb}")
                ps = pst[:, 0:512]
                for c in range(DC):
                    nc.tensor.matmul(
                        ps,
                        lhsT=xf[:, c, tbl * 128:(tbl + 1) * 128],
                        rhs=wv_sb[:, c, :],
                        start=(c == 0), stop=(c == DC - 1),
                    )
                for tt in range(4):
                    base = 130 * tt
                    nc.vector.tensor_copy(
                        vaug_sb[:, tb, base:base + 64],
                        ps[:, 128 * tt:128 * tt + 64])
                    nc.vector.tensor_copy(
                        vaug_sb[:, tb, base + 65:base + 129],
                        ps[:, 128 * tt + 64:128 * tt + 128])

        def own_K_and_kick():
            nc.sync.dma_start(out=wk_sb[:, :, 1:4, :], in_=wk[:, :, 1:4, :])
            wk23_loaded.append(True)
            for t in (2, 3):
                pst = psum.tile([128, 1024], f32, tag="s", name=f"psok{t}")
                ps = pst[:, 0:512]
                for c in range(DC):
                    nc.tensor.matmul(
                        ps, lhsT=wk_sb[:, c, t, :], rhs=xt_sb[:, c, :],
                        start=(c == 0), stop=(c == DC - 1),
                    )
                dst = kv_own[:, 512 * (t - 2):512 * (t - 1)]
                nc.vector.tensor_copy(dst, ps)
                rope(dst, cos_sb[:], sin_sb[:])
            kvd_in = dram.tile([128, 1024], bf16, tag="kvi", name="kvi")
            kvd_out = dram.tile([4, 128, 1024], bf16, tag="kvo", name="kvo")
            nc.gpsimd.dma_start(out=kvd_in[:], in_=kv_own[:])
            nc.gpsimd.collective_compute(
                "AllGather",
                mybir.AluOpType.bypass,
                replica_groups=[[0, 1, 2, 3], [4, 5, 6, 7]],
                ins=[kvd_in[:]],
                outs=[kvd_out[:]],
            )
            for j in range(4):
                nc.gpsimd.dma_start(out=kt_sb[:, 2:4, 512 * j:512 * j + 512],
                                    in_=kvd_out[j, :, 0:1024])

        def q_chain(fc, qt_sb):
            wq_t = wqp.tile([128, DC, 128], bf16, tag="wq", name=f"wqt{fc}")
            nc.sync.dma_start(out=wq_t[:], in_=wq[:, fc, :, :])
            pst = psum.tile([128, 1024], f32, tag="s", name=f"psq{fc}")
            ps = pst[:, 0:512]
            for c in range(DC):
                nc.tensor.matmul(
                    ps, lhsT=wq_t[:, c, :], rhs=xt_sb[:, c, :],
                    start=(c == 0), stop=(c == DC - 1),
                )
            nc.vector.tensor_copy(qt_sb[:, fc, :], ps)
            rope(qt_sb[:, fc, :], cos_sb[:], sin_sb[:])

        def attn_kb(t, fc, kb0, kb1, po, qt_sb):
            for kb in range(kb0, kb1):
                ksl = slice(kb * 128, (kb + 1) * 128)
                s = psum.tile([128, 1024], f32, tag="s", name=f"s{fc}_{kb}")
                nc.tensor.matmul(
                    s[:, 0:512], lhsT=kt_sb[0:64, t, ksl],
                    rhs=qt_sb[0:64, fc, :],
                    start=True, stop=True, tile_position=(0, 0),
                )
                nc.tensor.matmul(
                    s[:, 512:1024], lhsT=kt_sb[64:128, t, ksl],
                    rhs=qt_sb[64:128, fc, :],
                    start=True, stop=True, tile_position=(64, 0),
                )
                p = work.tile([128, 1024], bf16, tag="p", name=f"p{fc}_{kb}")
                nc.scalar.activation(p, s, Exp, scale=scale)
                nc.tensor.matmul(
                    po[:, 0:512], lhsT=vaug_sb[:, kb, 130 * t:130 * t + 65],
                    rhs=p[:, 0:512],
                    start=(kb == 0), stop=(kb == DC - 1),
                )
                nc.tensor.matmul(
                    po[:, 512:1024], lhsT=vaug_sb[:, kb, 130 * t + 65:130 * t + 130],
                    rhs=p[:, 512:1024],
                    start=(kb == 0), stop=(kb == DC - 1),
                )

        def attn_finish(fc, po):
            nc.vector.tensor_copy(ot_sb[0:64, fc, :], po[0:64, 0:512])
            nc.vector.tensor_copy(ot_sb[64:128, fc, :], po[0:64, 512:1024])
            dp = 32 * (fc % 4)
            df = (fc // 4) * 1024
            nc.vector.tensor_copy(dstage[dp:dp + 1, df:df + 1024],
                                  po[64:65, 0:1024])

        def attn_pair(t, r, qt_sb, interleave=()):
            fc = 4 * t + r
            po = apsum.tile([65, 1024], f32, tag="po", name=f"po{fc}")
            points = sorted(set(k for k, _ in interleave))
            cuts = [0] + points + [DC]
            for i in range(len(cuts) - 1):
                if i > 0:
                    for k, fn in interleave:
                        if k == cuts[i]:
                            fn()
                attn_kb(t, fc, cuts[i], cuts[i + 1], po, qt_sb)
            attn_finish(fc, po)

        def recip_quarter(t):
            hs = slice(1024 * t, 1024 * (t + 1))
            nc.vector.reciprocal_approx_fast(out=dstage[:, hs], in_=dstage[:, hs])
            for rr in range(4):
                nc.sync.dma_start(out=ddram[rr:rr + 1, hs],
                                  in_=dstage[32 * rr:32 * rr + 1, hs])
            for fc in range(4 * t, 4 * t + 4):
                dp = fc % 4
                df = (fc // 4) * 1024
                r2 = io.tile([128, TOK], bf16, tag="r2", name=f"r2_{fc}")
                nc.gpsimd.dma_start(
                    out=r2[0:64, :],
                    in_=ddram[dp:dp + 1, df:df + 512].partition_broadcast(64))
                nc.gpsimd.dma_start(
                    out=r2[64:128, :],
                    in_=ddram[dp:dp + 1, df + 512:df + 1024].partition_broadcast(64))
                nc.vector.tensor_mul(ot_sb[:, fc, :], ot_sb[:, fc, :], r2)

        # ================= schedule =================
        # round 0 pair 0 rides along pass A chunk by chunk
        qt_sb = consts.tile([128, DC, TOK], bf16, tag="qt")
        po0 = apsum.tile([65, 1024], f32, tag="po", name="po0")

        pass_K(0, 0, xfa0)
        q_chain(0, qt_sb)
        nc.sync.dma_start(out=wv_sb[:], in_=wv[:])  # queued after wq0
        pass_V(0, xfa0)
        attn_kb(0, 0, 0, 4, po0, qt_sb)

        xfa1 = xf_dma(1, "xfa1")
        pass_K(0, 1, xfa1)
        pass_V(1, xfa1)
        q_chain(1, qt_sb)
        attn_kb(0, 0, 4, 8, po0, qt_sb)

        own_K_and_kick()
        # ot reuses the kv_own slot (kv_own dead once kvd_in is written)
        ot_sb = consts.tile([128, DC, TOK], bf16, tag="kv")
        xfa2 = xf_dma(2, "xfa2")
        pass_K(0, 2, xfa2)
        pass_V(2, xfa2)
        q_chain(2, qt_sb)
        attn_kb(0, 0, 8, 12, po0, qt_sb)

        xfa3 = xf_dma(3, "xfa3")
        pass_K(0, 3, xfa3)
        pass_V(3, xfa3)
        q_chain(3, qt_sb)
        attn_kb(0, 0, 12, 16, po0, qt_sb)
        attn_finish(0, po0)

        def passB(ssl):
            def fn():
                xf = xf_dma(ssl, f"xfb{ssl}")
                pass_K(1, ssl, xf)
            return fn

        def qc(fc):
            return lambda: q_chain(fc, qt_sb)

        attn_pair(0, 1, qt_sb, interleave=((8, passB(0)),))
        attn_pair(0, 2, qt_sb, interleave=((8, passB(1)),))
        attn_pair(0, 3, qt_sb, interleave=((4, passB(2)), (10, qc(4))))
        recip_quarter(0)

        attn_pair(1, 0, qt_sb, interleave=((4, passB(3)), (10, qc(5))))
        attn_pair(1, 1, qt_sb, interleave=((8, qc(6)),))
        attn_pair(1, 2, qt_sb, interleave=((8, qc(7)),))
        attn_pair(1, 3, qt_sb, interleave=((8, qc(8)),))
        recip_quarter(1)

        attn_pair(2, 0, qt_sb, interleave=((8, qc(9)),))
        attn_pair(2, 1, qt_sb, interleave=((8, qc(10)),))
        attn_pair(2, 2, qt_sb, interleave=((8, qc(11)),))
        attn_pair(2, 3, qt_sb, interleave=((8, qc(12)),))
        recip_quarter(2)

        attn_pair(3, 0, qt_sb, interleave=((8, qc(13)),))
        attn_pair(3, 1, qt_sb, interleave=((8, qc(14)),))
        attn_pair(3, 2, qt_sb, interleave=((8, qc(15)),))
        attn_pair(3, 3, qt_sb)

        # prefetch the first two wo chunks before the tail chain
        wo_pre = []
        for fcg in range(2):
            wo_ch = wop.tile([128, 2, 1024], bf16, tag="woc", name=f"wopre{fcg}")
            nc.sync.dma_start(out=wo_ch[:], in_=wo[:, 2 * fcg:2 * fcg + 2, 0:1024])
            wo_pre.append(wo_ch)

        # warm-keepers bridge the final reciprocal chain (PE HAM clock)
        for i in range(6):
            sdum = psum.tile([128, 1024], f32, tag="s", name=f"sdum{i}")
            nc.tensor.matmul(
                sdum[0:65, 0:512], lhsT=vaug_sb[:, 0, 0:65],
                rhs=qt_sb[:, 0, :], start=True, stop=True,
            )
        recip_quarter(3)

        # ---- output projection: out[tok, D] = o_norm @ wo.T
        # 8 held accumulators (psum 2 tiles -> 4 halves, opsum 2, apsum 2
        # halves); wo streamed in [128, 2fc, 1024] chunks per column-half.
        for half in range(2):
            accs = []
            s_ts = []
            for i in range(3):
                s_t = psum.tile([128, 1024], f32, tag="s", name=f"oas{half}_{i}")
                s_ts.append(s_t)
                accs.append(s_t[:, 0:512])
                accs.append(s_t[:, 512:1024])
            po_t = apsum.tile([128, 1024], f32, tag="po", name=f"oap{half}")
            accs.append(po_t[:, 0:512])
            accs.append(po_t[:, 512:1024])
            # accs[tb2*2+dc2] covers out[tb2*128:+128, 1024*half+512*dc2:+512]
            for fcg in range(8):
                if half == 0 and fcg < 2:
                    wo_ch = wo_pre[fcg]
                else:
                    wo_ch = wop.tile([128, 2, 1024], bf16, tag="woc",
                                     name=f"woc{half}_{fcg}")
                    nc.sync.dma_start(
                        out=wo_ch[:],
                        in_=wo[:, 2 * fcg:2 * fcg + 2,
                               1024 * half:1024 * (half + 1)])
                for fl in range(2):
                    fc = 2 * fcg + fl
                    for tb2 in range(4):
                        tsl = slice(tb2 * 128, (tb2 + 1) * 128)
                        for dc2 in range(2):
                            nc.tensor.matmul(
                                accs[tb2 * 2 + dc2],
                                lhsT=ot_sb[:, fc, tsl],
                                rhs=wo_ch[:, fl, 512 * dc2:512 * (dc2 + 1)],
                                start=(fc == 0), stop=(fc == DC - 1),
                            )
            # evacuate: merged copies + 1024-wide DMAs
            for tb2, src_t in ((0, s_ts[0]), (1, s_ts[1]), (2, s_ts[2]),
                               (3, po_t)):
                osb = outw.tile([128, 1024], f32, tag="osb", name=f"ob{half}_{tb2}")
                nc.vector.tensor_copy(osb, src_t[:])
                nc.sync.dma_start(
                    out=out[tb2 * 128:(tb2 + 1) * 128,
                            1024 * half:1024 * (half + 1)],
                    in_=osb)

    nc.compile()
    return nc


def _prep_shared(freqs_cos, freqs_sin, wqkv, wo):
    """Weight/table prep shared by all cores."""
    cs = np.asarray(freqs_cos)[:, 0, :]  # [S, 64] (already repeat-2 layout)
    sn = np.asarray(freqs_sin)[:, 0, :]
    cos_h = np.empty((128, S), np.float32)
    sin_h = np.empty((128, S), np.float32)
    for p in range(128):
        cos_h[p] = cs[:, p % 64]
        sin_h[p] = sn[:, p % 64] * (-1.0 if p % 2 == 0 else 1.0)

    # Q rows permuted: fc = 4t+r -> [head 8t+r | head 8t+4+r]
    qrows = []
    for t in range(4):
        for r in range(4):
            for h in (8 * t + r, 8 * t + 4 + r):
                qrows.extend(range(h * HD, (h + 1) * HD))
    wq_t = np.ascontiguousarray(wqkv[qrows, :].T)  # [D, 2048]
    wq_h = np.ascontiguousarray(
        wq_t.reshape(DC, 128, DC, 128).transpose(1, 2, 0, 3)).astype(BF16)

    # K rows: tile t holds groups (2t | 2t+1)
    krows = []
    for t in range(4):
        for g in (2 * t, 2 * t + 1):
            krows.extend(range(H * HD + g * HD, H * HD + (g + 1) * HD))
    wk_t = np.ascontiguousarray(wqkv[krows, :].T)  # [D, 512]
    wk_h = np.ascontiguousarray(
        wk_t.reshape(DC, 128, 4, 128).transpose(1, 0, 2, 3)).astype(BF16)

    # V rows natural group order (cols t*128 : A 64 | B 64)
    vrows = list(range((H + G) * HD, (H + 2 * G) * HD))
    wv_t = np.ascontiguousarray(wqkv[vrows, :].T)  # [D, 512]
    wv_h = np.ascontiguousarray(
        wv_t.reshape(DC, 128, 512).transpose(1, 0, 2)).astype(BF16)

    # wo rhs: wo_h[p, fc, dcol] = wo[dcol, feat(fc, p)]
    feat = np.empty(D, np.int64)
    for fc in range(DC):
        t, r = divmod(fc, 4)
        for p in range(128):
            h = 8 * t + r + (4 if p >= 64 else 0)
            feat[fc * 128 + p] = h * HD + (p % 64)
    wo_h = np.ascontiguousarray(
        np.asarray(wo)[:, feat].T.reshape(DC, 128, D).transpose(1, 0, 2)
    ).astype(BF16)
    return cos_h, sin_h, wq_h, wk_h, wv_h, wo_h


def _prep_inputs(x, freqs_cos, freqs_sin, wqkv, wo):
    cos_h, sin_h, wq_h, wk_h, wv_h, wo_h = _prep_shared(
        freqs_cos, freqs_sin, wqkv, wo)
    x = np.asarray(x)
    cosf_h = np.ascontiguousarray(cos_h).astype(BF16)
    sinf_h = np.ascontiguousarray(sin_h).astype(BF16)
    xtf_hs = [
        np.ascontiguousarray(
            x[b].T.reshape(DC, 128, S).transpose(1, 0, 2)).astype(BF16)
        for b in range(B)
    ]
    ins = []
    for c in range(N_CORES):
        b, t4 = divmod(c, 4)
        q0 = t4 * TOK
        sl = slice(q0, q0 + TOK)
        xt_h = np.ascontiguousarray(xtf_hs[b][:, :, sl])
        ins.append({
            "xt": xt_h, "xtf": xtf_hs[b],
            "wq": wq_h, "wk": wk_h, "wv": wv_h, "wo": wo_h,
            "coso": np.ascontiguousarray(cos_h[:, sl]).astype(BF16),
            "sino": np.ascontiguousarray(sin_h[:, sl]).astype(BF16),
            "cosf": cosf_h, "sinf": sinf_h,
        })
    return ins


TRACE = False


def kernel(x, freqs_cos, freqs_sin, wqkv, wo):
    if "nc" not in _CACHE:
        _CACHE["nc"] = _build()
    nc = _CACHE["nc"]
    ins = _prep_inputs(x, freqs_cos, freqs_sin, wqkv, wo)
    res = run_bass_kernel_spmd(nc, ins, list(range(N_CORES)), trace=TRACE)
    _CACHE["res"] = res
    out = np.empty((B, S, D), np.float32)
    for c in range(N_CORES):
        b, t4 = divmod(c, 4)
        out[b, t4 * TOK:(t4 + 1) * TOK, :] = res.results[c]["out"]
    return out


if __name__ == "__main__":
    rng = np.random.default_rng(0)
    x = rng.normal(size=(B, S, D)).astype(np.float32)
    fc_ = rng.random(size=(S, 1, HD)).astype(np.float32)
    fs_ = rng.random(size=(S, 1, HD)).astype(np.float32)
    wq_ = rng.normal(size=(3072, D)).astype(np.float32) * 0.02
    wo_ = rng.normal(size=(D, D)).astype(np.float32) * 0.02
    o = kernel(x, fc_, fs_, wq_, wo_)
    print(o.shape, o.dtype)


# revision 12
# speedup vs baseline: 1.0564x; 1.0564x over previous
"""GQA attention (B=2, S=2048, D=2048, H=32, G=8, hd=64) on 8 TRN2 cores.

ZERO-COLLECTIVE sharding: core c owns (batch b=c//4, token block q0=512*(c%4)).
Each core computes the FULL output slice out[b, q0:q0+512, :] independently:
full K/V over all S (kv projection replicated within a batch group), Q only
for its own 512 tokens, attention + output projection fully local. No
cross-core communication, no barriers -> per-core span is immune to
cross-core start skew.

Per-core SPMD uniformity: the token axis of x^T (and the RoPE tables) is
ROTATED by q0 on the host, so "own tokens" are always columns 0:512.
Attention is permutation-invariant over keys; RoPE phases ride with the
rotation.

Layouts (host-side transposes):
  - x^T resident in SBUF [128, 16, S]; K^T [128(2 groups), 4, S] and
    V [tok-part, feat] projected on-chip; V is stored augmented with a ones
    column per head so the PV matmul accumulates the softmax denominator in
    PSUM row 64 for free. 1/d via exp(-ln(d)), folded into o^T before the
    output projection.
  - scores come out transposed [k, q] so PV needs no on-chip transposes.
  - head pair (hA=8t+r, hB=8t+4+r) processed together: one [128,1024] PSUM
    scores tile (A|B), one exp() activation covers both heads.
"""

import sys

sys.path.insert(0, "/opt/trn_rl_repo")

import numpy as np
import ml_dtypes

import concourse.bass as bass
import concourse.tile as tile
from concourse import bacc, mybir
from concourse.bass_utils import run_bass_kernel_spmd

BF16 = ml_dtypes.bfloat16
B, S, D = 2, 2048, 2048
H, G, HD = 32, 8, 64
DC = D // 128  # 16 dim chunks
N_CORES = 8
TOK = 512  # own tokens per core

_CACHE = {}


def _build():
    f32 = mybir.dt.float32
    bf16 = mybir.dt.bfloat16
    nc = bacc.Bacc("TRN2", target_bir_lowering=False, debug=False, num_devices=N_CORES)

    xt = nc.dram_tensor("xt", [128, DC, S], bf16, kind="ExternalInput").ap()
    wq = nc.dram_tensor("wq", [128, DC, DC, 128], bf16, kind="ExternalInput").ap()
    wk = nc.dram_tensor("wk", [128, DC, 4, 128], bf16, kind="ExternalInput").ap()
    wv = nc.dram_tensor("wv", [128, DC, 512], bf16, kind="ExternalInput").ap()
    cosr = nc.dram_tensor("cosr", [128, S], bf16, kind="ExternalInput").ap()
    sinr = nc.dram_tensor("sinr", [128, S], bf16, kind="ExternalInput").ap()
    wo = nc.dram_tensor("wo", [128, DC, D], bf16, kind="ExternalInput").ap()
    out = nc.dram_tensor("out", [TOK, D], f32, kind="ExternalOutput").ap()

    Exp = mybir.ActivationFunctionType.Exp
    Ln = mybir.ActivationFunctionType.Ln
    swap_mask = [i ^ 1 for i in range(32)]
    scale = float(1.0 / np.sqrt(HD))

    from contextlib import ExitStack
    with tile.TileContext(nc) as tc, ExitStack() as ctx:
        consts = ctx.enter_context(tc.tile_pool(name="consts", bufs=1))
        wqp = ctx.enter_context(tc.tile_pool(name="wqp", bufs=2))
        io = ctx.enter_context(tc.tile_pool(name="io", bufs=2))
        work = ctx.enter_context(tc.tile_pool(name="work", bufs=3))
        outw = ctx.enter_context(tc.tile_pool(name="outw", bufs=2))
        # scores get their own 2x[128,1024] pool (4 banks); projection
        # accumulators + PV accumulators + outproj share a 4x[*,512] pool
        # (4 banks) so attention overlaps the projections.
        psum = ctx.enter_context(tc.tile_pool(name="psum", bufs=2, space="PSUM"))
        opsum = ctx.enter_context(tc.tile_pool(name="opsum", bufs=4, space="PSUM"))
        dram = ctx.enter_context(tc.tile_pool(name="dram", bufs=1, space="DRAM"))

        # ---- load inputs, ordered for the earliest possible first vproj
        # matmul: wv + x^T chunks feed it; wk/cos/sin follow
        wv_sb = consts.tile([128, DC, 512], bf16, tag="wv")
        nc.sync.dma_start(out=wv_sb[:], in_=wv[:])
        xt_sb = consts.tile([128, DC, S], bf16, tag="xt")
        nc.sync.dma_start(out=xt_sb[:, 0:2, :], in_=xt[:, 0:2, :])
        wk_sb = consts.tile([128, DC, 4, 128], bf16, tag="wk")
        nc.sync.dma_start(out=wk_sb[:], in_=wk[:])
        for i in range(1, 8):
            nc.sync.dma_start(out=xt_sb[:, 2 * i:2 * i + 2, :],
                              in_=xt[:, 2 * i:2 * i + 2, :])
        cos_sb = consts.tile([128, S], bf16, tag="cos")
        nc.sync.dma_start(out=cos_sb[:], in_=cosr[:])
        sin_sb = consts.tile([128, S], bf16, tag="sin")
        nc.sync.dma_start(out=sin_sb[:], in_=sinr[:])

        kt_sb = consts.tile([128, 4, S], bf16, tag="kt")
        vaug_sb = consts.tile([128, DC, 520], bf16, tag="vaug")
        qt_sb = consts.tile([128, DC, TOK], bf16, tag="qt")
        # denominator staging: pair fc -> partition 32*(fc%4), free block fc//4
        # (DVE partition bases must be 32-aligned). memset(1) keeps the unused
        # rows finite through the Ln/Exp pass.
        dstage = consts.tile([97, 4096], f32, tag="dstage")
        nc.vector.memset(dstage[:], 1.0)
        ddram = dram.tile([4, 4096], f32, tag="dd", name="dd")

        # ones columns of augmented V (130t+64 for head A, 130t+129 for B)
        for t in range(4):
            nc.vector.memset(vaug_sb[:, :, 130 * t + 64:130 * t + 65], 1.0)
            nc.vector.memset(vaug_sb[:, :, 130 * t + 129:130 * t + 130], 1.0)

        def rope(ap, cs, sn):
            sw = io.tile([128, 512], bf16, tag="rsw")
            nc.vector.stream_shuffle(sw, ap, swap_mask)
            nc.vector.tensor_mul(sw, sw, sn)
            tmp = io.tile([128, 512], bf16, tag="rtmp")
            nc.vector.tensor_mul(tmp, ap, cs)
            nc.vector.tensor_add(ap, sw, tmp)

        # ---- V projection into augmented layout (first: attention streams it)
        for tb in range(DC):
            ps = opsum.tile([128, 512], f32, tag="o")
            for c in range(DC):
                nc.tensor.matmul(
                    ps,
                    lhsT=xt_sb[:, c, tb * 128:(tb + 1) * 128],
                    rhs=wv_sb[:, c, :],
                    start=(c == 0), stop=(c == DC - 1),
                )
            for t in range(4):
                nc.vector.tensor_copy(
                    vaug_sb[:, tb, 130 * t:130 * t + 64],
                    ps[:, t * 128:t * 128 + 64])
                nc.vector.tensor_copy(
                    vaug_sb[:, tb, 130 * t + 65:130 * t + 129],
                    ps[:, t * 128 + 64:t * 128 + 128])

        # ot reuses the wv slot (wv dead after vproj)
        ot_sb = consts.tile([128, DC, TOK], bf16, tag="wv")

        def recip_quarter(t):
            # 1/d for pairs fc in [4t, 4t+4): dstage free cols [1024t, 1024t+1024)
            hs = slice(1024 * t, 1024 * (t + 1))
            nc.scalar.activation(dstage[:, hs], dstage[:, hs], Ln)
            nc.scalar.activation(dstage[:, hs], dstage[:, hs], Exp, scale=-1.0)
            for rr in range(4):
                nc.sync.dma_start(out=ddram[rr:rr + 1, hs],
                                  in_=dstage[32 * rr:32 * rr + 1, hs])
            for fc in range(4 * t, 4 * t + 4):
                dp = fc % 4
                df = (fc // 4) * 1024
                r2 = io.tile([128, TOK], bf16, tag="r2")
                nc.gpsimd.dma_start(
                    out=r2[0:64, :],
                    in_=ddram[dp:dp + 1, df:df + 512].partition_broadcast(64))
                nc.gpsimd.dma_start(
                    out=r2[64:128, :],
                    in_=ddram[dp:dp + 1, df + 512:df + 1024].partition_broadcast(64))
                nc.vector.tensor_mul(ot_sb[:, fc, :], ot_sb[:, fc, :], r2)

        # ---- rounds: projections emitted ONE ROUND AHEAD of the attention
        # that consumes them, so TensorE's slack during ACT-bound attention
        # prefetches the next round and exp() never waits at round boundaries.
        def proj_round(t):
            for ssl in range(4):
                # one K chain, then one Q chain: the K-RoPE DVE latency
                # hides under the next chain's matmuls
                sl = slice(ssl * 512, (ssl + 1) * 512)
                ps = opsum.tile([128, 512], f32, tag="o")
                for c in range(DC):
                    nc.tensor.matmul(
                        ps,
                        lhsT=wk_sb[:, c, t, :],
                        rhs=xt_sb[:, c, sl],
                        start=(c == 0), stop=(c == DC - 1),
                    )
                nc.vector.tensor_copy(kt_sb[:, t, sl], ps)
                rope(kt_sb[:, t, sl], cos_sb[:, sl], sin_sb[:, sl])
                fc = 4 * t + ssl
                wq_t = wqp.tile([128, DC, 128], bf16, tag="wq")
                nc.sync.dma_start(out=wq_t[:], in_=wq[:, fc, :, :])
                ps = opsum.tile([128, 512], f32, tag="o")
                for c in range(DC):
                    nc.tensor.matmul(
                        ps,
                        lhsT=wq_t[:, c, :],
                        rhs=xt_sb[:, c, 0:TOK],
                        start=(c == 0), stop=(c == DC - 1),
                    )
                nc.vector.tensor_copy(qt_sb[:, fc, :], ps)
                rope(qt_sb[:, fc, :], cos_sb[:, 0:TOK], sin_sb[:, 0:TOK])

        def attn_round(t, r0, r1):
            # attention pairs of this t: heads (8t+r | 8t+4+r), fc = 4t+r
            for r in range(r0, r1):
                fc = 4 * t + r
                oA = opsum.tile([65, 512], f32, tag="o")
                oB = opsum.tile([65, 512], f32, tag="o")
                for kb in range(DC):
                    ksl = slice(kb * 128, (kb + 1) * 128)
                    s = psum.tile([128, 1024], f32, tag="s")
                    nc.tensor.matmul(
                        s[:, 0:512], lhsT=kt_sb[0:64, t, ksl],
                        rhs=qt_sb[0:64, fc, :],
                        start=True, stop=True, tile_position=(0, 0),
                    )
                    nc.tensor.matmul(
                        s[:, 512:1024], lhsT=kt_sb[64:128, t, ksl],
                        rhs=qt_sb[64:128, fc, :],
                        start=True, stop=True, tile_position=(64, 0),
                    )
                    p = work.tile([128, 1024], bf16, tag="p")
                    nc.scalar.activation(p, s, Exp, scale=scale)
                    nc.tensor.matmul(
                        oA, lhsT=vaug_sb[:, kb, 130 * t:130 * t + 65],
                        rhs=p[:, 0:512],
                        start=(kb == 0), stop=(kb == DC - 1),
                    )
                    nc.tensor.matmul(
                        oB, lhsT=vaug_sb[:, kb, 130 * t + 65:130 * t + 130],
                        rhs=p[:, 512:1024],
                        start=(kb == 0), stop=(kb == DC - 1),
                    )
                nc.vector.tensor_copy(ot_sb[0:64, fc, :], oA[0:64, :])
                nc.vector.tensor_copy(ot_sb[64:128, fc, :], oB[0:64, :])
                dp = 32 * (fc % 4)
                df = (fc // 4) * 1024
                nc.vector.tensor_copy(
                    dstage[dp:dp + 1, df:df + 512], oA[64:65, :])
                nc.vector.tensor_copy(
                    dstage[dp:dp + 1, df + 512:df + 1024], oB[64:65, :])


        proj_round(0)
        attn_round(0, 0, 2)
        proj_round(1)
        attn_round(0, 2, 4)
        recip_quarter(0)
        proj_round(2)
        attn_round(1, 0, 4)
        recip_quarter(1)
        proj_round(3)
        # wo reuses the xt slot (xt dead after the last projections)
        wo_sb = consts.tile([128, DC, D], bf16, tag="xt")
        for i in range(4):
            nc.sync.dma_start(out=wo_sb[:, 4 * i:4 * i + 4, :],
                              in_=wo[:, 4 * i:4 * i + 4, :])
        attn_round(2, 0, 4)
        recip_quarter(2)
        attn_round(3, 0, 3)
        # last quarter's reciprocals for pairs r=0..2 early (rows 0/32/64),
        # so only row 96 remains after the final pair
        hs3 = slice(3072, 4096)
        nc.scalar.activation(dstage[0:65, hs3], dstage[0:65, hs3], Ln)
        nc.scalar.activation(dstage[0:65, hs3], dstage[0:65, hs3], Exp,
                             scale=-1.0)
        for rr in range(3):
            nc.sync.dma_start(out=ddram[rr:rr + 1, hs3],
                              in_=dstage[32 * rr:32 * rr + 1, hs3])
        for fc in (12, 13, 14):
            df = (fc // 4) * 1024
            r2 = io.tile([128, TOK], bf16, tag="r2")
            nc.gpsimd.dma_start(
                out=r2[0:64, :],
                in_=ddram[fc % 4:fc % 4 + 1, df:df + 512].partition_broadcast(64))
            nc.gpsimd.dma_start(
                out=r2[64:128, :],
                in_=ddram[fc % 4:fc % 4 + 1, df + 512:df + 1024].partition_broadcast(64))
            nc.vector.tensor_mul(ot_sb[:, fc, :], ot_sb[:, fc, :], r2)
        attn_round(3, 3, 4)
        # warm-keepers: harmless matmuls bridge the final reciprocal chain so
        # the PE HAM clock stays at 8/8 and the output projection starts warm
        for _ in range(12):
            sdum = psum.tile([128, 1024], f32, tag="s")
            nc.tensor.matmul(
                sdum[0:65, 0:512], lhsT=vaug_sb[:, 0, 0:65],
                rhs=qt_sb[:, 0, :], start=True, stop=True,
            )
        nc.scalar.activation(dstage[96:97, hs3], dstage[96:97, hs3], Ln)
        nc.scalar.activation(dstage[96:97, hs3], dstage[96:97, hs3], Exp,
                             scale=-1.0)
        nc.sync.dma_start(out=ddram[3:4, hs3], in_=dstage[96:97, hs3])
        r2 = io.tile([128, TOK], bf16, tag="r2")
        nc.gpsimd.dma_start(
            out=r2[0:64, :],
            in_=ddram[3:4, 3072:3584].partition_broadcast(64))
        nc.gpsimd.dma_start(
            out=r2[64:128, :],
            in_=ddram[3:4, 3584:4096].partition_broadcast(64))
        nc.vector.tensor_mul(ot_sb[:, 15, :], ot_sb[:, 15, :], r2)

        # ---- output projection: out[tok, D] = o_norm @ wo.T
        for tb2 in range(4):
            tsl = slice(tb2 * 128, (tb2 + 1) * 128)
            for dc in range(4):
                dsl = slice(dc * 512, (dc + 1) * 512)
                ps = opsum.tile([128, 512], f32, tag="o")
                for fc in range(DC):
                    nc.tensor.matmul(
                        ps,
                        lhsT=ot_sb[:, fc, tsl],
                        rhs=wo_sb[:, fc, dsl],
                        start=(fc == 0), stop=(fc == DC - 1),
                    )
                osb = outw.tile([128, 512], f32, tag="osb")
                nc.vector.tensor_copy(osb, ps)
                nc.sync.dma_start(out=out[tsl, dsl], in_=osb)

    nc.compile()
    return nc


def _prep_shared(freqs_cos, freqs_sin, wqkv, wo):
    """Weight/table prep shared by all cores (token rotation applied later)."""
    cs = np.asarray(freqs_cos)[:, 0, :]  # [S, 64] (already repeat-2 layout)
    sn = np.asarray(freqs_sin)[:, 0, :]
    cos_h = np.empty((128, S), np.float32)
    sin_h = np.empty((128, S), np.float32)
    for p in range(128):
        cos_h[p] = cs[:, p % 64]
        sin_h[p] = sn[:, p % 64] * (-1.0 if p % 2 == 0 else 1.0)

    # Q rows permuted: fc = 4t+r -> [head 8t+r | head 8t+4+r]
    qrows = []
    for t in range(4):
        for r in range(4):
            for h in (8 * t + r, 8 * t + 4 + r):
                qrows.extend(range(h * HD, (h + 1) * HD))
    wq_t = np.ascontiguousarray(wqkv[qrows, :].T)  # [D, 2048]
    wq_h = np.ascontiguousarray(
        wq_t.reshape(DC, 128, DC, 128).transpose(1, 2, 0, 3)).astype(BF16)

    # K rows: tile t holds groups (2t | 2t+1)
    krows = []
    for t in range(4):
        for g in (2 * t, 2 * t + 1):
            krows.extend(range(H * HD + g * HD, H * HD + (g + 1) * HD))
    wk_t = np.ascontiguousarray(wqkv[krows, :].T)  # [D, 512]
    wk_h = np.ascontiguousarray(
        wk_t.reshape(DC, 128, 4, 128).transpose(1, 0, 2, 3)).astype(BF16)

    # V rows natural group order (cols t*128 : A 64 | B 64)
    vrows = list(range((H + G) * HD, (H + 2 * G) * HD))
    wv_t = np.ascontiguousarray(wqkv[vrows, :].T)  # [D, 512]
    wv_h = np.ascontiguousarray(
        wv_t.reshape(DC, 128, 512).transpose(1, 0, 2)).astype(BF16)

    # wo rhs: wo_h[p, fc, dcol] = wo[dcol, feat(fc, p)]
    feat = np.empty(D, np.int64)
    for fc in range(DC):
        t, r = divmod(fc, 4)
        for p in range(128):
            h = 8 * t + r + (4 if p >= 64 else 0)
            feat[fc * 128 + p] = h * HD + (p % 64)
    wo_h = np.ascontiguousarray(
        np.asarray(wo)[:, feat].T.reshape(DC, 128, D).transpose(1, 0, 2)
    ).astype(BF16)
    return cos_h, sin_h, wq_h, wk_h, wv_h, wo_h


def _prep_inputs(x, freqs_cos, freqs_sin, wqkv, wo):
    cos_h, sin_h, wq_h, wk_h, wv_h, wo_h = _prep_shared(
        freqs_cos, freqs_sin, wqkv, wo)
    x = np.asarray(x)
    ins = []
    for c in range(N_CORES):
        b, t4 = divmod(c, 4)
        q0 = t4 * TOK
        rot = (np.arange(S) + q0) % S  # own tokens land at cols 0:512
        xt_h = np.ascontiguousarray(
            x[b].T[:, rot].reshape(DC, 128, S).transpose(1, 0, 2)).astype(BF16)
        ins.append({
            "xt": xt_h,
            "wq": wq_h, "wk": wk_h, "wv": wv_h, "wo": wo_h,
            "cosr": np.ascontiguousarray(cos_h[:, rot]).astype(BF16),
            "sinr": np.ascontiguousarray(sin_h[:, rot]).astype(BF16),
        })
    return ins


TRACE = False


def kernel(x, freqs_cos, freqs_sin, wqkv, wo):
    if "nc" not in _CACHE:
        _CACHE["nc"] = _build()
    nc = _CACHE["nc"]
    ins = _prep_inputs(x, freqs_cos, freqs_sin, wqkv, wo)
    res = run_bass_kernel_spmd(nc, ins, list(range(N_CORES)), trace=TRACE)
    _CACHE["res"] = res
    out = np.empty((B, S, D), np.float32)
    for c in range(N_CORES):
        b, t4 = divmod(c, 4)
        out[b, t4 * TOK:(t4 + 1) * TOK, :] = res.results[c]["out"]
    return out


if __name__ == "__main__":
    rng = np.random.default_rng(0)
    x = rng.normal(size=(B, S, D)).astype(np.float32)
    fc_ = rng.random(size=(S, 1, HD)).astype(np.float32)
    fs_ = rng.random(size=(S, 1, HD)).astype(np.float32)
    wq_ = rng.normal(size=(3072, D)).astype(np.float32) * 0.02
    wo_ = rng.normal(size=(D, D)).astype(np.float32) * 0.02
    o = kernel(x, fc_, fs_, wq_, wo_)
    print(o.shape, o.dtype)



# revision 13
# speedup vs baseline: 1.1012x; 1.0424x over previous
"""GQA attention (B=2, S=2048, D=2048, H=32, G=8, hd=64) on 8 TRN2 cores.

ZERO-COLLECTIVE sharding: core c owns (batch b=c//4, token block
q0=512*(c%4)). Each core computes the FULL output slice out[b,
q0:q0+512, :] independently: full K/V over all S (replicated within a
batch group), Q only for its own 512 tokens, attention + output
projection fully local. Collectives are deliberately avoided: on this
runtime an AllGather trips a GPIO power throttle that caps the PE clock
at 81% for most of the kernel (~+100us) on top of ~60-100us latency.

Per-core SPMD uniformity: the token axis of x^T (and the RoPE tables) is
ROTATED by q0 on the host, so "own tokens" are always columns 0:512.
Attention is permutation-invariant over keys; RoPE phases ride with the
rotation.

Schedule: x^T streams in 512-token chunks; attention round 0 pair 0
rides the first pass chunk-by-chunk (K tiles 0,1 + V + Q per chunk), so
the exp stream starts at ~40us instead of waiting for the full 8.4 MB
x^T load. K tiles 2,3 and remaining Q chains interleave into later
pairs' PE slack, keeping the PE dense (HAM clock stays 8/8). Softmax
denominators accumulate free in PSUM row 64 via an augmented-V ones
column; reciprocals run on DVE (reciprocal_approx_fast) so ACT does
pure exp with a single table load (preheated).

PSUM: scores 2x[128,1024] (4 banks) + chain accumulators 2x[128,512]
(2) + PV accumulator [65,1024] (2, heads A|B side by side). The output
projection reuses all 8 banks as 8 held accumulators while wo streams
through SBUF in 0.5 MB chunks.
"""

import sys

sys.path.insert(0, "/opt/trn_rl_repo")

import numpy as np
import ml_dtypes

import concourse.bass as bass
import concourse.tile as tile
from concourse import bacc, mybir
from concourse.bass_utils import run_bass_kernel_spmd

BF16 = ml_dtypes.bfloat16
B, S, D = 2, 2048, 2048
H, G, HD = 32, 8, 64
DC = D // 128  # 16 dim chunks
N_CORES = 8
TOK = 512  # own tokens per core

_CACHE = {}


def _build():
    f32 = mybir.dt.float32
    bf16 = mybir.dt.bfloat16
    nc = bacc.Bacc("TRN2", target_bir_lowering=False, debug=False, num_devices=N_CORES)

    xt = nc.dram_tensor("xt", [128, DC, S], bf16, kind="ExternalInput").ap()
    wq = nc.dram_tensor("wq", [128, DC, DC, 128], bf16, kind="ExternalInput").ap()
    wk = nc.dram_tensor("wk", [128, DC, 4, 128], bf16, kind="ExternalInput").ap()
    wv = nc.dram_tensor("wv", [128, DC, 512], bf16, kind="ExternalInput").ap()
    cosr = nc.dram_tensor("cosr", [128, S], bf16, kind="ExternalInput").ap()
    sinr = nc.dram_tensor("sinr", [128, S], bf16, kind="ExternalInput").ap()
    wo = nc.dram_tensor("wo", [128, DC, D], bf16, kind="ExternalInput").ap()
    out = nc.dram_tensor("out", [TOK, D], f32, kind="ExternalOutput").ap()

    Exp = mybir.ActivationFunctionType.Exp
    swap_mask = [i ^ 1 for i in range(32)]
    scale = float(1.0 / np.sqrt(HD))

    from contextlib import ExitStack
    with tile.TileContext(nc) as tc, ExitStack() as ctx:
        consts = ctx.enter_context(tc.tile_pool(name="consts", bufs=1))
        xtfp = ctx.enter_context(tc.tile_pool(name="xtfp", bufs=2))
        wqp = ctx.enter_context(tc.tile_pool(name="wqp", bufs=2))
        wop = ctx.enter_context(tc.tile_pool(name="wop", bufs=2))
        io = ctx.enter_context(tc.tile_pool(name="io", bufs=2))
        work = ctx.enter_context(tc.tile_pool(name="work", bufs=3))
        outw = ctx.enter_context(tc.tile_pool(name="outw", bufs=2))
        psum = ctx.enter_context(tc.tile_pool(name="psum", bufs=2, space="PSUM"))
        opsum = ctx.enter_context(tc.tile_pool(name="opsum", bufs=2, space="PSUM"))
        apsum = ctx.enter_context(tc.tile_pool(name="apsum", bufs=1, space="PSUM"))
        dram = ctx.enter_context(tc.tile_pool(name="dram", bufs=1, space="DRAM"))

        # ---- header loads, ordered to unlock K-chunk0 -> Q0 -> V-chunk0.
        # xf0 (own tokens) is persistent: all 16 Q chains read it.
        wk_sb = consts.tile([128, DC, 4, 128], bf16, tag="wk")
        nc.sync.dma_start(out=wk_sb[:, :, 0:2, :], in_=wk[:, :, 0:2, :])
        xf0 = consts.tile([128, DC, 512], bf16, tag="xf0")
        nc.sync.dma_start(out=xf0[:], in_=xt[:, :, 0:512])
        cosf_sb = consts.tile([128, S], bf16, tag="cosf")
        nc.sync.dma_start(out=cosf_sb[:], in_=cosr[:])
        sinf_sb = consts.tile([128, S], bf16, tag="sinf")
        nc.sync.dma_start(out=sinf_sb[:], in_=sinr[:])
        # wq0 is emitted by q_chain(0) below -> lands here in queue order
        wv_sb = consts.tile([128, DC, 512], bf16, tag="wv")

        # preheat the exp table set (one ACT_TABLE_LOAD, early)
        preheat = io.tile([1, 64], bf16, tag="pre")
        nc.vector.memset(preheat[:], 0.0)
        nc.scalar.activation(preheat[:], preheat[:], Exp)

        kt_sb = consts.tile([128, 4, S], bf16, tag="kt")
        vaug_sb = consts.tile([128, DC, 520], bf16, tag="vaug")
        qt_sb = consts.tile([128, DC, TOK], bf16, tag="qt")
        ot_sb = consts.tile([128, DC, TOK], bf16, tag="ot")
        dstage = consts.tile([97, 4096], f32, tag="dstage")
        nc.vector.memset(dstage[:], 1.0)
        ddram = dram.tile([4, 4096], f32, tag="dd", name="dd")
        # ones columns of augmented V (denominator accumulators)
        for col in (64, 129, 194, 259, 324, 389, 454, 519):
            nc.vector.memset(vaug_sb[:, :, col:col + 1], 1.0)

        def rope(ap, cs, sn):
            sw = io.tile([128, 512], bf16, tag="rsw")
            nc.vector.stream_shuffle(sw, ap, swap_mask)
            nc.vector.tensor_mul(sw, sw, sn)
            tmp = io.tile([128, 512], bf16, tag="rtmp")
            nc.vector.tensor_mul(tmp, ap, cs)
            nc.vector.tensor_add(ap, sw, tmp)

        def xf_dma(ssl, name):
            xf = xtfp.tile([128, DC, 512], bf16, tag="xf", name=name)
            nc.sync.dma_start(out=xf[:], in_=xt[:, :, 512 * ssl:512 * (ssl + 1)])
            return xf

        def pass_K(t, ssl, xf):
            sl = slice(512 * ssl, 512 * (ssl + 1))
            ps = opsum.tile([128, 512], f32, tag="o", name=f"psk{t}_{ssl}")
            for c in range(DC):
                nc.tensor.matmul(
                    ps, lhsT=wk_sb[:, c, t, :], rhs=xf[:, c, :],
                    start=(c == 0), stop=(c == DC - 1),
                )
            nc.vector.tensor_copy(kt_sb[:, t, sl], ps)
            rope(kt_sb[:, t, sl], cosf_sb[:, sl], sinf_sb[:, sl])

        def pass_V(ssl, xf):
            # full V projection for 512 tokens -> vaug cols 0:520
            for tbl in range(4):
                tb = 4 * ssl + tbl
                ps = opsum.tile([128, 512], f32, tag="o", name=f"psv{tb}")
                for c in range(DC):
                    nc.tensor.matmul(
                        ps,
                        lhsT=xf[:, c, tbl * 128:(tbl + 1) * 128],
                        rhs=wv_sb[:, c, :],
                        start=(c == 0), stop=(c == DC - 1),
                    )
                for tt in range(4):
                    base = 130 * tt
                    nc.vector.tensor_copy(
                        vaug_sb[:, tb, base:base + 64],
                        ps[:, 128 * tt:128 * tt + 64])
                    nc.vector.tensor_copy(
                        vaug_sb[:, tb, base + 65:base + 129],
                        ps[:, 128 * tt + 64:128 * tt + 128])

        def q_chain(fc):
            wq_t = wqp.tile([128, DC, 128], bf16, tag="wq", name=f"wqt{fc}")
            nc.sync.dma_start(out=wq_t[:], in_=wq[:, fc, :, :])
            ps = opsum.tile([128, 512], f32, tag="o", name=f"psq{fc}")
            for c in range(DC):
                nc.tensor.matmul(
                    ps, lhsT=wq_t[:, c, :], rhs=xf0[:, c, :],
                    start=(c == 0), stop=(c == DC - 1),
                )
            nc.vector.tensor_copy(qt_sb[:, fc, :], ps)
            rope(qt_sb[:, fc, :], cosf_sb[:, 0:512], sinf_sb[:, 0:512])

        def attn_kb(t, fc, kb0, kb1, po):
            for kb in range(kb0, kb1):
                ksl = slice(kb * 128, (kb + 1) * 128)
                s = psum.tile([128, 1024], f32, tag="s", name=f"s{fc}_{kb}")
                nc.tensor.matmul(
                    s[:, 0:512], lhsT=kt_sb[0:64, t, ksl],
                    rhs=qt_sb[0:64, fc, :],
                    start=True, stop=True, tile_position=(0, 0),
                )
                nc.tensor.matmul(
                    s[:, 512:1024], lhsT=kt_sb[64:128, t, ksl],
                    rhs=qt_sb[64:128, fc, :],
                    start=True, stop=True, tile_position=(64, 0),
                )
                p = work.tile([128, 1024], bf16, tag="p", name=f"p{fc}_{kb}")
                nc.scalar.activation(p, s, Exp, scale=scale)
                nc.tensor.matmul(
                    po[:, 0:512], lhsT=vaug_sb[:, kb, 130 * t:130 * t + 65],
                    rhs=p[:, 0:512],
                    start=(kb == 0), stop=(kb == DC - 1),
                )
                nc.tensor.matmul(
                    po[:, 512:1024], lhsT=vaug_sb[:, kb, 130 * t + 65:130 * t + 130],
                    rhs=p[:, 512:1024],
                    start=(kb == 0), stop=(kb == DC - 1),
                )

        def attn_finish(fc, po):
            nc.vector.tensor_copy(ot_sb[0:64, fc, :], po[0:64, 0:512])
            nc.vector.tensor_copy(ot_sb[64:128, fc, :], po[0:64, 512:1024])
            dp = 32 * (fc % 4)
            df = (fc // 4) * 1024
            nc.vector.tensor_copy(dstage[dp:dp + 1, df:df + 1024],
                                  po[64:65, 0:1024])

        def attn_pair(t, r, interleave=()):
            fc = 4 * t + r
            po = apsum.tile([65, 1024], f32, tag="po", name=f"po{fc}")
            points = sorted(set(k for k, _ in interleave))
            cuts = [0] + points + [DC]
            for i in range(len(cuts) - 1):
                if i > 0:
                    for k, fn in interleave:
                        if k == cuts[i]:
                            fn()
                attn_kb(t, fc, cuts[i], cuts[i + 1], po)
            attn_finish(fc, po)

        def recip_quarter(t):
            hs = slice(1024 * t, 1024 * (t + 1))
            nc.vector.reciprocal_approx_fast(out=dstage[:, hs], in_=dstage[:, hs])
            for rr in range(4):
                nc.sync.dma_start(out=ddram[rr:rr + 1, hs],
                                  in_=dstage[32 * rr:32 * rr + 1, hs])
            for fc in range(4 * t, 4 * t + 4):
                dp = fc % 4
                df = (fc // 4) * 1024
                r2 = io.tile([128, TOK], bf16, tag="r2", name=f"r2_{fc}")
                nc.gpsimd.dma_start(
                    out=r2[0:64, :],
                    in_=ddram[dp:dp + 1, df:df + 512].partition_broadcast(64))
                nc.gpsimd.dma_start(
                    out=r2[64:128, :],
                    in_=ddram[dp:dp + 1, df + 512:df + 1024].partition_broadcast(64))
                nc.vector.tensor_mul(ot_sb[:, fc, :], ot_sb[:, fc, :], r2)

        def kpass(t, ssl):
            def fn():
                xf = xf_dma(ssl, f"xfk{t}_{ssl}")
                pass_K(t, ssl, xf)
            return fn

        def qc(fc):
            return lambda: q_chain(fc)

        # ================= schedule =================
        # round 0 pair 0 rides the first streaming pass chunk by chunk:
        # per 512-token chunk: K tiles 0,1 + V + one Q chain + 4 kb of
        # attention. Own tokens are chunk 0 (rotated layout).
        po0 = apsum.tile([65, 1024], f32, tag="po", name="po0")

        pass_K(0, 0, xf0)
        q_chain(0)
        nc.sync.dma_start(out=wv_sb[:], in_=wv[:])  # queued after wq0
        pass_K(1, 0, xf0)
        pass_V(0, xf0)
        attn_kb(0, 0, 0, 4, po0)

        for ssl in (1, 2, 3):
            xf = xf_dma(ssl, f"xfa{ssl}")
            pass_K(0, ssl, xf)
            pass_K(1, ssl, xf)
            pass_V(ssl, xf)
            q_chain(ssl)
            attn_kb(0, 0, 4 * ssl, 4 * (ssl + 1), po0)
        attn_finish(0, po0)
        # rest of wk (tiles 2,3) for the later K passes
        nc.sync.dma_start(out=wk_sb[:, :, 2:4, :], in_=wk[:, :, 2:4, :])

        attn_pair(0, 1, interleave=((8, kpass(2, 0)),))
        attn_pair(0, 2, interleave=((8, kpass(2, 1)),))
        attn_pair(0, 3, interleave=((4, kpass(2, 2)), (10, qc(4))))
        recip_quarter(0)

        attn_pair(1, 0, interleave=((4, kpass(2, 3)), (10, qc(5))))
        attn_pair(1, 1, interleave=((4, kpass(3, 0)), (10, qc(6))))
        attn_pair(1, 2, interleave=((4, kpass(3, 1)), (10, qc(7))))
        attn_pair(1, 3, interleave=((4, kpass(3, 2)), (10, qc(8))))
        recip_quarter(1)

        attn_pair(2, 0, interleave=((4, kpass(3, 3)), (10, qc(9))))
        attn_pair(2, 1, interleave=((8, qc(10)),))
        attn_pair(2, 2, interleave=((8, qc(11)),))
        attn_pair(2, 3, interleave=((8, qc(12)),))
        recip_quarter(2)

        attn_pair(3, 0, interleave=((8, qc(13)),))
        attn_pair(3, 1, interleave=((8, qc(14)),))
        attn_pair(3, 2, interleave=((8, qc(15)),))
        attn_pair(3, 3)

        # prefetch the first two wo chunks before the tail chain
        wo_pre = []
        for fcg in range(2):
            wo_ch = wop.tile([128, 2, 1024], bf16, tag="woc", name=f"wopre{fcg}")
            nc.sync.dma_start(out=wo_ch[:], in_=wo[:, 2 * fcg:2 * fcg + 2, 0:1024])
            wo_pre.append(wo_ch)

        # warm-keepers bridge the final reciprocal chain (PE HAM clock)
        for i in range(8):
            sdum = psum.tile([128, 1024], f32, tag="s", name=f"sdum{i}")
            nc.tensor.matmul(
                sdum[0:65, 0:512], lhsT=vaug_sb[:, 0, 0:65],
                rhs=qt_sb[:, 0, :], start=True, stop=True,
            )
        recip_quarter(3)

        # ---- output projection: out[tok, D] = o_norm @ wo.T
        # 8 held accumulators: psum 2 tiles -> 4 halves, opsum 2, apsum
        # [128,1024] -> 2 halves; wo streamed in [128, 2fc, 1024] chunks.
        for half in range(2):
            accs = []
            s_ts = []
            for i in range(2):
                s_t = psum.tile([128, 1024], f32, tag="s", name=f"oas{half}_{i}")
                s_ts.append(s_t)
                accs.append(s_t[:, 0:512])
                accs.append(s_t[:, 512:1024])
            o_ts = [opsum.tile([128, 512], f32, tag="o", name=f"oao{half}_{i}")
                    for i in range(2)]
            accs.extend(o_ts)
            po_t = apsum.tile([128, 1024], f32, tag="po", name=f"oap{half}")
            accs.append(po_t[:, 0:512])
            accs.append(po_t[:, 512:1024])
            # accs[tb2*2+dc2] covers out[tb2*128:+128, 1024*half+512*dc2:+512]
            for fcg in range(8):
                if half == 0 and fcg < 2:
                    wo_ch = wo_pre[fcg]
                else:
                    wo_ch = wop.tile([128, 2, 1024], bf16, tag="woc",
                                     name=f"woc{half}_{fcg}")
                    nc.sync.dma_start(
                        out=wo_ch[:],
                        in_=wo[:, 2 * fcg:2 * fcg + 2,
                               1024 * half:1024 * (half + 1)])
                for fl in range(2):
                    fc = 2 * fcg + fl
                    for tb2 in range(4):
                        tsl = slice(tb2 * 128, (tb2 + 1) * 128)
                        for dc2 in range(2):
                            nc.tensor.matmul(
                                accs[tb2 * 2 + dc2],
                                lhsT=ot_sb[:, fc, tsl],
                                rhs=wo_ch[:, fl, 512 * dc2:512 * (dc2 + 1)],
                                start=(fc == 0), stop=(fc == DC - 1),
                            )
            # evacuate: merged copies + 1024-wide DMAs
            for tb2, src_t in ((0, s_ts[0][:]), (1, s_ts[1][:]), (3, po_t[:])):
                osb = outw.tile([128, 1024], f32, tag="osb",
                                name=f"ob{half}_{tb2}")
                nc.vector.tensor_copy(osb, src_t)
                nc.sync.dma_start(
                    out=out[tb2 * 128:(tb2 + 1) * 128,
                            1024 * half:1024 * (half + 1)],
                    in_=osb)
            osb = outw.tile([128, 1024], f32, tag="osb", name=f"ob{half}_2")
            nc.vector.tensor_copy(osb[:, 0:512], o_ts[0])
            nc.vector.tensor_copy(osb[:, 512:1024], o_ts[1])
            nc.sync.dma_start(
                out=out[256:384, 1024 * half:1024 * (half + 1)], in_=osb)

    nc.compile()
    return nc


def _prep_shared(freqs_cos, freqs_sin, wqkv, wo):
    """Weight/table prep shared by all cores (token rotation applied later)."""
    cs = np.asarray(freqs_cos)[:, 0, :]  # [S, 64] (already repeat-2 layout)
    sn = np.asarray(freqs_sin)[:, 0, :]
    cos_h = np.empty((128, S), np.float32)
    sin_h = np.empty((128, S), np.float32)
    for p in range(128):
        cos_h[p] = cs[:, p % 64]
        sin_h[p] = sn[:, p % 64] * (-1.0 if p % 2 == 0 else 1.0)

    # Q rows permuted: fc = 4t+r -> [head 8t+r | head 8t+4+r]
    qrows = []
    for t in range(4):
        for r in range(4):
            for h in (8 * t + r, 8 * t + 4 + r):
                qrows.extend(range(h * HD, (h + 1) * HD))
    wq_t = np.ascontiguousarray(wqkv[qrows, :].T)  # [D, 2048]
    wq_h = np.ascontiguousarray(
        wq_t.reshape(DC, 128, DC, 128).transpose(1, 2, 0, 3)).astype(BF16)

    # K rows: tile t holds groups (2t | 2t+1)
    krows = []
    for t in range(4):
        for g in (2 * t, 2 * t + 1):
            krows.extend(range(H * HD + g * HD, H * HD + (g + 1) * HD))
    wk_t = np.ascontiguousarray(wqkv[krows, :].T)  # [D, 512]
    wk_h = np.ascontiguousarray(
        wk_t.reshape(DC, 128, 4, 128).transpose(1, 0, 2, 3)).astype(BF16)

    # V rows natural group order (cols t*128 : A 64 | B 64)
    vrows = list(range((H + G) * HD, (H + 2 * G) * HD))
    wv_t = np.ascontiguousarray(wqkv[vrows, :].T)  # [D, 512]
    wv_h = np.ascontiguousarray(
        wv_t.reshape(DC, 128, 512).transpose(1, 0, 2)).astype(BF16)

    # wo rhs: wo_h[p, fc, dcol] = wo[dcol, feat(fc, p)]
    feat = np.empty(D, np.int64)
    for fc in range(DC):
        t, r = divmod(fc, 4)
        for p in range(128):
            h = 8 * t + r + (4 if p >= 64 else 0)
            feat[fc * 128 + p] = h * HD + (p % 64)
    wo_h = np.ascontiguousarray(
        np.asarray(wo)[:, feat].T.reshape(DC, 128, D).transpose(1, 0, 2)
    ).astype(BF16)
    return cos_h, sin_h, wq_h, wk_h, wv_h, wo_h


def _prep_inputs(x, freqs_cos, freqs_sin, wqkv, wo):
    cos_h, sin_h, wq_h, wk_h, wv_h, wo_h = _prep_shared(
        freqs_cos, freqs_sin, wqkv, wo)
    x = np.asarray(x)
    ins = []
    for c in range(N_CORES):
        b, t4 = divmod(c, 4)
        q0 = t4 * TOK
        rot = (np.arange(S) + q0) % S  # own tokens land at cols 0:512
        xt_h = np.ascontiguousarray(
            x[b].T[:, rot].reshape(DC, 128, S).transpose(1, 0, 2)).astype(BF16)
        ins.append({
            "xt": xt_h,
            "wq": wq_h, "wk": wk_h, "wv": wv_h, "wo": wo_h,
            "cosr": np.ascontiguousarray(cos_h[:, rot]).astype(BF16),
            "sinr": np.ascontiguousarray(sin_h[:, rot]).astype(BF16),
        })
    return ins


TRACE = False


def kernel(x, freqs_cos, freqs_sin, wqkv, wo):
    if "nc" not in _CACHE:
        _CACHE["nc"] = _build()
    nc = _CACHE["nc"]
    ins = _prep_inputs(x, freqs_cos, freqs_sin, wqkv, wo)
    res = run_bass_kernel_spmd(nc, ins, list(range(N_CORES)), trace=TRACE)
    _CACHE["res"] = res
    out = np.empty((B, S, D), np.float32)
    for c in range(N_CORES):
        b, t4 = divmod(c, 4)
        out[b, t4 * TOK:(t4 + 1) * TOK, :] = res.results[c]["out"]
    return out


if __name__ == "__main__":
    rng = np.random.default_rng(0)
    x = rng.normal(size=(B, S, D)).astype(np.float32)
    fc_ = rng.random(size=(S, 1, HD)).astype(np.float32)
    fs_ = rng.random(size=(S, 1, HD)).astype(np.float32)
    wq_ = rng.normal(size=(3072, D)).astype(np.float32) * 0.02
    wo_ = rng.normal(size=(D, D)).astype(np.float32) * 0.02
    o = kernel(x, fc_, fs_, wq_, wo_)
    print(o.shape, o.dtype)


# revision 14
# speedup vs baseline: 1.1455x; 1.0402x over previous
"""GQA attention (B=2, S=2048, D=2048, H=32, G=8, hd=64) on 8 TRN2 cores.

ZERO-COLLECTIVE sharding: core c owns (batch b=c//4, token block
q0=512*(c%4)). Each core computes the FULL output slice out[b,
q0:q0+512, :] independently: full K/V over all S (replicated within a
batch group), Q only for its own 512 tokens, attention + output
projection fully local. Collectives are deliberately avoided: on this
runtime an AllGather trips a GPIO power throttle that caps the PE clock
at 81% for most of the kernel (~+100us) on top of ~60-100us latency.

Per-core SPMD uniformity: the token axis of x^T (and the RoPE tables) is
ROTATED by q0 on the host, so "own tokens" are always columns 0:512.
Attention is permutation-invariant over keys; RoPE phases ride with the
rotation.

Schedule: x^T streams in 512-token chunks; attention round 0 pair 0
rides the first pass chunk-by-chunk (K tiles 0,1 + V + Q per chunk), so
the exp stream starts at ~40us instead of waiting for the full 8.4 MB
x^T load. K tiles 2,3 and remaining Q chains interleave into later
pairs' PE slack, keeping the PE dense (HAM clock stays 8/8). Softmax
denominators accumulate free in PSUM row 64 via an augmented-V ones
column; reciprocals run on DVE (reciprocal_approx_fast) so ACT does
pure exp with a single table load (preheated).

PSUM: scores 2x[128,1024] (4 banks) + chain accumulators 2x[128,512]
(2) + PV accumulator [65,1024] (2, heads A|B side by side). The output
projection reuses all 8 banks as 8 held accumulators while wo streams
through SBUF in 0.5 MB chunks.
"""

import sys

sys.path.insert(0, "/opt/trn_rl_repo")

import numpy as np
import ml_dtypes

import concourse.bass as bass
import concourse.tile as tile
from concourse import bacc, mybir
from concourse.bass_utils import run_bass_kernel_spmd

BF16 = ml_dtypes.bfloat16
B, S, D = 2, 2048, 2048
H, G, HD = 32, 8, 64
DC = D // 128  # 16 dim chunks
N_CORES = 8
TOK = 512  # own tokens per core

_CACHE = {}


def _build():
    f32 = mybir.dt.float32
    bf16 = mybir.dt.bfloat16
    nc = bacc.Bacc("TRN2", target_bir_lowering=False, debug=False, num_devices=N_CORES)

    xt = nc.dram_tensor("xt", [128, DC, S], bf16, kind="ExternalInput").ap()
    wq = nc.dram_tensor("wq", [128, DC, DC, 128], bf16, kind="ExternalInput").ap()
    wk = nc.dram_tensor("wk", [128, DC, 4, 128], bf16, kind="ExternalInput").ap()
    wv = nc.dram_tensor("wv", [128, DC, 512], bf16, kind="ExternalInput").ap()
    cosr = nc.dram_tensor("cosr", [128, S], bf16, kind="ExternalInput").ap()
    sinr = nc.dram_tensor("sinr", [128, S], bf16, kind="ExternalInput").ap()
    wo = nc.dram_tensor("wo", [128, DC, D], bf16, kind="ExternalInput").ap()
    out = nc.dram_tensor("out", [TOK, D], f32, kind="ExternalOutput").ap()

    Exp = mybir.ActivationFunctionType.Exp
    swap_mask = [i ^ 1 for i in range(32)]
    scale = float(1.0 / np.sqrt(HD))

    from contextlib import ExitStack
    with tile.TileContext(nc) as tc, ExitStack() as ctx:
        consts = ctx.enter_context(tc.tile_pool(name="consts", bufs=1))
        xtfp = ctx.enter_context(tc.tile_pool(name="xtfp", bufs=2))
        wqp = ctx.enter_context(tc.tile_pool(name="wqp", bufs=2))
        io = ctx.enter_context(tc.tile_pool(name="io", bufs=2))
        work = ctx.enter_context(tc.tile_pool(name="work", bufs=3))
        outw = ctx.enter_context(tc.tile_pool(name="outw", bufs=2))
        psum = ctx.enter_context(tc.tile_pool(name="psum", bufs=2, space="PSUM"))
        opsum = ctx.enter_context(tc.tile_pool(name="opsum", bufs=2, space="PSUM"))
        apsum = ctx.enter_context(tc.tile_pool(name="apsum", bufs=1, space="PSUM"))
        dram = ctx.enter_context(tc.tile_pool(name="dram", bufs=1, space="DRAM"))

        # ---- header loads, ordered to unlock K-chunk0 -> Q0 -> V-chunk0.
        # xf0 (own tokens) is persistent: all 16 Q chains read it.
        wk_sb = consts.tile([128, DC, 4, 128], bf16, tag="wk")
        nc.sync.dma_start(out=wk_sb[:, :, 0:1, :], in_=wk[:, :, 0:1, :])
        xf0 = consts.tile([128, DC, 512], bf16, tag="xf0")
        nc.sync.dma_start(out=xf0[:], in_=xt[:, :, 0:512])
        nc.sync.dma_start(out=wk_sb[:, :, 1:2, :], in_=wk[:, :, 1:2, :])
        cosf_sb = consts.tile([128, S], bf16, tag="cosf")
        nc.sync.dma_start(out=cosf_sb[:], in_=cosr[:])
        sinf_sb = consts.tile([128, S], bf16, tag="sinf")
        nc.sync.dma_start(out=sinf_sb[:], in_=sinr[:])
        # wq0 is emitted by q_chain(0) below -> lands here in queue order
        wv_sb = consts.tile([128, DC, 512], bf16, tag="wv")

        # preheat the exp table set (one ACT_TABLE_LOAD, early)
        preheat = io.tile([1, 64], bf16, tag="pre")
        nc.vector.memset(preheat[:], 0.0)
        nc.scalar.activation(preheat[:], preheat[:], Exp)

        kt_sb = consts.tile([128, 4, S], bf16, tag="kt")
        vaug_sb = consts.tile([128, DC, 520], bf16, tag="vaug")
        qt_sb = consts.tile([128, DC, TOK], bf16, tag="qt")
        ot_sb = consts.tile([128, DC, TOK], bf16, tag="ot")
        dstage = consts.tile([97, 4096], f32, tag="dstage")
        nc.vector.memset(dstage[:], 1.0)
        ddram = dram.tile([4, 4096], f32, tag="dd", name="dd")
        # ones columns of augmented V (denominator accumulators)
        for col in (64, 129, 194, 259, 324, 389, 454, 519):
            nc.vector.memset(vaug_sb[:, :, col:col + 1], 1.0)

        def rope(ap, cs, sn):
            sw = io.tile([128, 512], bf16, tag="rsw")
            nc.vector.stream_shuffle(sw, ap, swap_mask)
            nc.vector.tensor_mul(sw, sw, sn)
            tmp = io.tile([128, 512], bf16, tag="rtmp")
            nc.vector.tensor_mul(tmp, ap, cs)
            nc.vector.tensor_add(ap, sw, tmp)

        def xf_dma(ssl, name):
            xf = xtfp.tile([128, DC, 512], bf16, tag="xf", name=name)
            nc.sync.dma_start(out=xf[:], in_=xt[:, :, 512 * ssl:512 * (ssl + 1)])
            return xf

        def pass_K(t, ssl, xf):
            sl = slice(512 * ssl, 512 * (ssl + 1))
            ps = opsum.tile([128, 512], f32, tag="o", name=f"psk{t}_{ssl}")
            for c in range(DC):
                nc.tensor.matmul(
                    ps, lhsT=wk_sb[:, c, t, :], rhs=xf[:, c, :],
                    start=(c == 0), stop=(c == DC - 1),
                )
            nc.vector.tensor_copy(kt_sb[:, t, sl], ps)
            rope(kt_sb[:, t, sl], cosf_sb[:, sl], sinf_sb[:, sl])

        def pass_V(ssl, xf):
            # full V projection for 512 tokens -> vaug cols 0:520
            for tbl in range(4):
                tb = 4 * ssl + tbl
                ps = opsum.tile([128, 512], f32, tag="o", name=f"psv{tb}")
                for c in range(DC):
                    nc.tensor.matmul(
                        ps,
                        lhsT=xf[:, c, tbl * 128:(tbl + 1) * 128],
                        rhs=wv_sb[:, c, :],
                        start=(c == 0), stop=(c == DC - 1),
                    )
                for tt in range(4):
                    base = 130 * tt
                    nc.vector.tensor_copy(
                        vaug_sb[:, tb, base:base + 64],
                        ps[:, 128 * tt:128 * tt + 64])
                    nc.vector.tensor_copy(
                        vaug_sb[:, tb, base + 65:base + 129],
                        ps[:, 128 * tt + 64:128 * tt + 128])

        def q_chain(fc):
            wq_t = wqp.tile([128, DC, 128], bf16, tag="wq", name=f"wqt{fc}")
            nc.sync.dma_start(out=wq_t[:], in_=wq[:, fc, :, :])
            ps = opsum.tile([128, 512], f32, tag="o", name=f"psq{fc}")
            for c in range(DC):
                nc.tensor.matmul(
                    ps, lhsT=wq_t[:, c, :], rhs=xf0[:, c, :],
                    start=(c == 0), stop=(c == DC - 1),
                )
            nc.vector.tensor_copy(qt_sb[:, fc, :], ps)
            rope(qt_sb[:, fc, :], cosf_sb[:, 0:512], sinf_sb[:, 0:512])

        def attn_kb(t, fc, kb0, kb1, po):
            for kb in range(kb0, kb1):
                ksl = slice(kb * 128, (kb + 1) * 128)
                s = psum.tile([128, 1024], f32, tag="s", name=f"s{fc}_{kb}")
                nc.tensor.matmul(
                    s[:, 0:512], lhsT=kt_sb[0:64, t, ksl],
                    rhs=qt_sb[0:64, fc, :],
                    start=True, stop=True, tile_position=(0, 0),
                )
                nc.tensor.matmul(
                    s[:, 512:1024], lhsT=kt_sb[64:128, t, ksl],
                    rhs=qt_sb[64:128, fc, :],
                    start=True, stop=True, tile_position=(64, 0),
                )
                p = work.tile([128, 1024], bf16, tag="p", name=f"p{fc}_{kb}")
                nc.scalar.activation(p, s, Exp, scale=scale)
                nc.tensor.matmul(
                    po[:, 0:512], lhsT=vaug_sb[:, kb, 130 * t:130 * t + 65],
                    rhs=p[:, 0:512],
                    start=(kb == 0), stop=(kb == DC - 1),
                )
                nc.tensor.matmul(
                    po[:, 512:1024], lhsT=vaug_sb[:, kb, 130 * t + 65:130 * t + 130],
                    rhs=p[:, 512:1024],
                    start=(kb == 0), stop=(kb == DC - 1),
                )

        def attn_finish(fc, po):
            nc.vector.tensor_copy(ot_sb[0:64, fc, :], po[0:64, 0:512])
            nc.vector.tensor_copy(ot_sb[64:128, fc, :], po[0:64, 512:1024])
            dp = 32 * (fc % 4)
            df = (fc // 4) * 1024
            nc.vector.tensor_copy(dstage[dp:dp + 1, df:df + 1024],
                                  po[64:65, 0:1024])

        def attn_pair(t, r, interleave=()):
            fc = 4 * t + r
            po = apsum.tile([65, 1024], f32, tag="po", name=f"po{fc}")
            points = sorted(set(k for k, _ in interleave))
            cuts = [0] + points + [DC]
            for i in range(len(cuts) - 1):
                if i > 0:
                    for k, fn in interleave:
                        if k == cuts[i]:
                            fn()
                attn_kb(t, fc, cuts[i], cuts[i + 1], po)
            attn_finish(fc, po)

        def recip_core(t):
            hs = slice(1024 * t, 1024 * (t + 1))
            nc.vector.reciprocal_approx_fast(out=dstage[:, hs], in_=dstage[:, hs])
            for rr in range(4):
                nc.sync.dma_start(out=ddram[rr:rr + 1, hs],
                                  in_=dstage[32 * rr:32 * rr + 1, hs])

        def mul_fc(fc):
            # normalize ot chunk fc by 1/d (broadcast via DRAM bounce rows);
            # spread across later pairs so DVE never spikes at round edges
            def fn():
                dp = fc % 4
                df = (fc // 4) * 1024
                r2 = io.tile([128, TOK], bf16, tag="r2", name=f"r2_{fc}")
                nc.gpsimd.dma_start(
                    out=r2[0:64, :],
                    in_=ddram[dp:dp + 1, df:df + 512].partition_broadcast(64))
                nc.gpsimd.dma_start(
                    out=r2[64:128, :],
                    in_=ddram[dp:dp + 1, df + 512:df + 1024].partition_broadcast(64))
                nc.vector.tensor_mul(ot_sb[:, fc, :], ot_sb[:, fc, :], r2)
            return fn

        def kpass(t, ssl):
            def fn():
                xf = xf_dma(ssl, f"xfk{t}_{ssl}")
                pass_K(t, ssl, xf)
            return fn

        def qc(fc):
            return lambda: q_chain(fc)

        # ================= schedule =================
        # round 0 pair 0 rides the first streaming pass chunk by chunk:
        # per 512-token chunk: K tiles 0,1 + V + one Q chain + 4 kb of
        # attention. Own tokens are chunk 0 (rotated layout).
        po0 = apsum.tile([65, 1024], f32, tag="po", name="po0")

        pass_K(0, 0, xf0)
        q_chain(0)
        nc.sync.dma_start(out=wv_sb[:], in_=wv[:])  # queued after wq0
        pass_K(1, 0, xf0)
        pass_V(0, xf0)
        attn_kb(0, 0, 0, 4, po0)

        for ssl in (1, 2, 3):
            xf = xf_dma(ssl, f"xfa{ssl}")
            pass_K(0, ssl, xf)
            pass_K(1, ssl, xf)
            pass_V(ssl, xf)
            q_chain(ssl)
            attn_kb(0, 0, 4 * ssl, 4 * (ssl + 1), po0)
        attn_finish(0, po0)
        # rest of wk (tiles 2,3) for the later K passes
        nc.sync.dma_start(out=wk_sb[:, :, 2:4, :], in_=wk[:, :, 2:4, :])

        attn_pair(0, 1, interleave=((8, kpass(2, 0)),))
        attn_pair(0, 2, interleave=((8, kpass(2, 1)),))
        attn_pair(0, 3, interleave=((4, kpass(2, 2)), (10, qc(4))))
        recip_core(0)

        attn_pair(1, 0, interleave=((4, kpass(2, 3)), (8, qc(5)),
                                    (13, mul_fc(0))))
        attn_pair(1, 1, interleave=((4, kpass(3, 0)), (8, qc(6)),
                                    (13, mul_fc(1))))
        attn_pair(1, 2, interleave=((4, kpass(3, 1)), (8, qc(7)),
                                    (13, mul_fc(2))))
        attn_pair(1, 3, interleave=((4, kpass(3, 2)), (8, qc(8)),
                                    (13, mul_fc(3))))
        recip_core(1)

        attn_pair(2, 0, interleave=((4, kpass(3, 3)), (8, qc(9)),
                                    (13, mul_fc(4))))
        attn_pair(2, 1, interleave=((6, qc(10)), (13, mul_fc(5))))
        attn_pair(2, 2, interleave=((6, qc(11)), (13, mul_fc(6))))
        attn_pair(2, 3, interleave=((6, qc(12)), (13, mul_fc(7))))
        recip_core(2)

        attn_pair(3, 0, interleave=((6, qc(13)), (13, mul_fc(8))))
        attn_pair(3, 1, interleave=((6, qc(14)), (13, mul_fc(9))))
        attn_pair(3, 2, interleave=((6, qc(15)), (13, mul_fc(10))))
        attn_pair(3, 3, interleave=((13, mul_fc(11)),))

        # wo ring buffer reuses the dead wv slot: 4 chunks of
        # [128, 2, 1024], 4-deep prefetch via subtile WAR
        wo_ring = consts.tile([128, 8, 1024], bf16, tag="wv")

        def wo_dma(k, half, fcg):
            j = 2 * (k % 4)
            nc.sync.dma_start(
                out=wo_ring[:, j:j + 2, :],
                in_=wo[:, 2 * fcg:2 * fcg + 2, 1024 * half:1024 * (half + 1)])
            return wo_ring[:, j:j + 2, :]

        wo_pre = [wo_dma(0, 0, 0), wo_dma(1, 0, 1)]

        # warm-keepers bridge the final reciprocal chain (PE HAM clock)
        for i in range(8):
            sdum = psum.tile([128, 1024], f32, tag="s", name=f"sdum{i}")
            nc.tensor.matmul(
                sdum[0:65, 0:512], lhsT=vaug_sb[:, 0, 0:65],
                rhs=qt_sb[:, 0, :], start=True, stop=True,
            )
        recip_core(3)
        for fc in (12, 13, 14, 15):
            mul_fc(fc)()

        # ---- output projection: out[tok, D] = o_norm @ wo.T
        # 8 held accumulators: psum 2 tiles -> 4 halves, opsum 2, apsum
        # [128,1024] -> 2 halves; wo streamed in [128, 2fc, 1024] chunks.
        for half in range(2):
            accs = []
            s_ts = []
            for i in range(2):
                s_t = psum.tile([128, 1024], f32, tag="s", name=f"oas{half}_{i}")
                s_ts.append(s_t)
                accs.append(s_t[:, 0:512])
                accs.append(s_t[:, 512:1024])
            o_ts = [opsum.tile([128, 512], f32, tag="o", name=f"oao{half}_{i}")
                    for i in range(2)]
            accs.extend(o_ts)
            po_t = apsum.tile([128, 1024], f32, tag="po", name=f"oap{half}")
            accs.append(po_t[:, 0:512])
            accs.append(po_t[:, 512:1024])
            # accs[tb2*2+dc2] covers out[tb2*128:+128, 1024*half+512*dc2:+512]
            for fcg in range(8):
                k = half * 8 + fcg
                if k < 2:
                    wo_ch = wo_pre[k]
                else:
                    wo_ch = wo_dma(k, half, fcg)
                for fl in range(2):
                    fc = 2 * fcg + fl
                    for tb2 in range(4):
                        tsl = slice(tb2 * 128, (tb2 + 1) * 128)
                        for dc2 in range(2):
                            nc.tensor.matmul(
                                accs[tb2 * 2 + dc2],
                                lhsT=ot_sb[:, fc, tsl],
                                rhs=wo_ch[:, fl, 512 * dc2:512 * (dc2 + 1)],
                                start=(fc == 0), stop=(fc == DC - 1),
                            )
            # evacuate: merged copies + 1024-wide DMAs
            for tb2, src_t in ((0, s_ts[0][:]), (1, s_ts[1][:]), (3, po_t[:])):
                osb = outw.tile([128, 1024], f32, tag="osb",
                                name=f"ob{half}_{tb2}")
                nc.vector.tensor_copy(osb, src_t)
                nc.sync.dma_start(
                    out=out[tb2 * 128:(tb2 + 1) * 128,
                            1024 * half:1024 * (half + 1)],
                    in_=osb)
            osb = outw.tile([128, 1024], f32, tag="osb", name=f"ob{half}_2")
            nc.vector.tensor_copy(osb[:, 0:512], o_ts[0])
            nc.vector.tensor_copy(osb[:, 512:1024], o_ts[1])
            nc.sync.dma_start(
                out=out[256:384, 1024 * half:1024 * (half + 1)], in_=osb)

    nc.compile()
    return nc


def _prep_shared(freqs_cos, freqs_sin, wqkv, wo):
    """Weight/table prep shared by all cores (token rotation applied later)."""
    cs = np.asarray(freqs_cos)[:, 0, :]  # [S, 64] (already repeat-2 layout)
    sn = np.asarray(freqs_sin)[:, 0, :]
    cos_h = np.empty((128, S), np.float32)
    sin_h = np.empty((128, S), np.float32)
    for p in range(128):
        cos_h[p] = cs[:, p % 64]
        sin_h[p] = sn[:, p % 64] * (-1.0 if p % 2 == 0 else 1.0)

    # Q rows permuted: fc = 4t+r -> [head 8t+r | head 8t+4+r]
    qrows = []
    for t in range(4):
        for r in range(4):
            for h in (8 * t + r, 8 * t + 4 + r):
                qrows.extend(range(h * HD, (h + 1) * HD))
    wq_t = np.ascontiguousarray(wqkv[qrows, :].T)  # [D, 2048]
    wq_h = np.ascontiguousarray(
        wq_t.reshape(DC, 128, DC, 128).transpose(1, 2, 0, 3)).astype(BF16)

    # K rows: tile t holds groups (2t | 2t+1)
    krows = []
    for t in range(4):
        for g in (2 * t, 2 * t + 1):
            krows.extend(range(H * HD + g * HD, H * HD + (g + 1) * HD))
    wk_t = np.ascontiguousarray(wqkv[krows, :].T)  # [D, 512]
    wk_h = np.ascontiguousarray(
        wk_t.reshape(DC, 128, 4, 128).transpose(1, 0, 2, 3)).astype(BF16)

    # V rows natural group order (cols t*128 : A 64 | B 64)
    vrows = list(range((H + G) * HD, (H + 2 * G) * HD))
    wv_t = np.ascontiguousarray(wqkv[vrows, :].T)  # [D, 512]
    wv_h = np.ascontiguousarray(
        wv_t.reshape(DC, 128, 512).transpose(1, 0, 2)).astype(BF16)

    # wo rhs: wo_h[p, fc, dcol] = wo[dcol, feat(fc, p)]
    feat = np.empty(D, np.int64)
    for fc in range(DC):
        t, r = divmod(fc, 4)
        for p in range(128):
            h = 8 * t + r + (4 if p >= 64 else 0)
            feat[fc * 128 + p] = h * HD + (p % 64)
    wo_h = np.ascontiguousarray(
        np.asarray(wo)[:, feat].T.reshape(DC, 128, D).transpose(1, 0, 2)
    ).astype(BF16)
    return cos_h, sin_h, wq_h, wk_h, wv_h, wo_h


def _prep_inputs(x, freqs_cos, freqs_sin, wqkv, wo):
    cos_h, sin_h, wq_h, wk_h, wv_h, wo_h = _prep_shared(
        freqs_cos, freqs_sin, wqkv, wo)
    x = np.asarray(x)
    ins = []
    for c in range(N_CORES):
        b, t4 = divmod(c, 4)
        q0 = t4 * TOK
        rot = (np.arange(S) + q0) % S  # own tokens land at cols 0:512
        xt_h = np.ascontiguousarray(
            x[b].T[:, rot].reshape(DC, 128, S).transpose(1, 0, 2)).astype(BF16)
        ins.append({
            "xt": xt_h,
            "wq": wq_h, "wk": wk_h, "wv": wv_h, "wo": wo_h,
            "cosr": np.ascontiguousarray(cos_h[:, rot]).astype(BF16),
            "sinr": np.ascontiguousarray(sin_h[:, rot]).astype(BF16),
        })
    return ins


TRACE = False


def kernel(x, freqs_cos, freqs_sin, wqkv, wo):
    if "nc" not in _CACHE:
        _CACHE["nc"] = _build()
    nc = _CACHE["nc"]
    ins = _prep_inputs(x, freqs_cos, freqs_sin, wqkv, wo)
    res = run_bass_kernel_spmd(nc, ins, list(range(N_CORES)), trace=TRACE)
    _CACHE["res"] = res
    out = np.empty((B, S, D), np.float32)
    for c in range(N_CORES):
        b, t4 = divmod(c, 4)
        out[b, t4 * TOK:(t4 + 1) * TOK, :] = res.results[c]["out"]
    return out


if __name__ == "__main__":
    rng = np.random.default_rng(0)
    x = rng.normal(size=(B, S, D)).astype(np.float32)
    fc_ = rng.random(size=(S, 1, HD)).astype(np.float32)
    fs_ = rng.random(size=(S, 1, HD)).astype(np.float32)
    wq_ = rng.normal(size=(3072, D)).astype(np.float32) * 0.02
    wo_ = rng.normal(size=(D, D)).astype(np.float32) * 0.02
    o = kernel(x, fc_, fs_, wq_, wo_)
    print(o.shape, o.dtype)
